# revision 1
# baseline (speedup 1.0000x reference)
"""Trainium2 Bass kernel for nn_Attn_30623116820602.

Low-rank-projected causal multi-head attention:
  q/k/v = (x @ A) @ B  (rank 192), RoPE on q,k, causal softmax attention,
  output projection.  x: [128, 256, 768] fp32.

Sharding: pure data-parallel over batch (16 items per core, 8 cores).
Device layout is feature-major (d_model on partitions) throughout, so no
on-chip transposes are needed; the host pre-transposes x per core and
post-transposes the result.  RoPE's rotate-half is pre-folded into extra
weight matrices (qB_rot/kB_rot) so the tensor engine emits both q and
rot(q); softmax runs with keys on partitions (no max subtraction --
scores are ~N(0,1)), denominators come from ones-vector matmuls, and the
per-query normalization is broadcast via gpsimd and fused into the
PSUM->SBUF copy of the attention output.
"""

import math
import sys

sys.path.insert(0, "/opt/trn_rl_repo")

import numpy as np
import ml_dtypes


def _to_bf16(a):
    return a.astype(ml_dtypes.bfloat16)


B, T, D = 128, 256, 768
H, HD = 6, 128
RANK = 192  # padded to 256 on host
N_CORES = 8
B_LOC = B // N_CORES  # 16
N_PAIRS = B_LOC // 2  # 8 (2 batch items per pipeline iteration)
SCALE = 1.0 / math.sqrt(HD)

_CACHE = {}


def build_program(n_pairs=N_PAIRS):
    import concourse.tile as tile
    from concourse import bacc, mybir
    from contextlib import ExitStack

    f32 = mybir.dt.float32
    f32r = mybir.dt.float32r
    TOK = n_pairs * 512


    nc = bacc.Bacc("TRN2", target_bir_lowering=False, debug=False,
                   num_devices=N_CORES)

    def din(name, shape):
        return nc.dram_tensor(name, shape, f32, kind="ExternalInput").ap()

    xT = din("xT", [6, 128, TOK])
    qA_l, kA_l, vA_l = (din(n, [6, 128, 192]) for n in ("qA_l", "kA_l", "vA_l"))
    qB_l, qBr_l, kB_l, kBr_l, vB_l = (
        din(n, [2, 128, 768]) for n in ("qB_l", "qBr_l", "kB_l", "kBr_l", "vB_l"))
    ow_l = din("ow_l", [6, 128, 768])
    cos2 = din("cos2", [128, 512])
    sin2 = din("sin2", [128, 512])
    mask_bf = nc.dram_tensor("mask_bf", [128, 512], mybir.dt.bfloat16, kind="ExternalInput").ap()
    onec = din("onec", [128, 1])
    outT = nc.dram_tensor("outT", [6, 128, TOK], f32, kind="ExternalOutput").ap()

    with tile.TileContext(nc) as tc:
        with ExitStack() as ctx:
            wp = ctx.enter_context(tc.tile_pool(name="w", bufs=1))
            xp = ctx.enter_context(tc.tile_pool(name="xt", bufs=2))
            xrp = ctx.enter_context(tc.tile_pool(name="xr", bufs=1))
            qkp = ctx.enter_context(tc.tile_pool(name="qk", bufs=1))
            tp = ctx.enter_context(tc.tile_pool(name="tmp", bufs=2))
            ep = ctx.enter_context(tc.tile_pool(name="eexp", bufs=3))
            dp = ctx.enter_context(tc.tile_pool(name="den", bufs=2))
            bp = ctx.enter_context(tc.tile_pool(name="bcast", bufs=1))
            aop = ctx.enter_context(tc.tile_pool(name="ao", bufs=2))
            fp = ctx.enter_context(tc.tile_pool(name="fout", bufs=1))
            ddrp = ctx.enter_context(tc.tile_pool(name="ddr", bufs=2, space="DRAM"))
            ps = ctx.enter_context(tc.tile_pool(name="ps", bufs=5, space="PSUM"))
            psa = ctx.enter_context(tc.tile_pool(name="psa", bufs=3, space="PSUM"))

            def psum():
                return ps.tile([128, 512], f32, tag="ps", name="psb")

            def psumA():
                return psa.tile([128, 512], f32, tag="psa", name="psa")

            # ---- resident weights / constants ----
            def wload(name, src, shape, perm):
                t = wp.tile(shape, f32r, tag=name, name=name)
                nc.gpsimd.dma_start(t[:], src.rearrange(perm).bitcast(f32r))
                return t

            qA_s = wload("qA", qA_l, [128, 6, 192], "k p m -> p k m")
            kA_s = wload("kA", kA_l, [128, 6, 192], "k p m -> p k m")
            vA_s = wload("vA", vA_l, [128, 6, 192], "k p m -> p k m")
            qB_s = wload("qB", qB_l, [128, 2, 768], "k p m -> p k m")
            qBr_s = wload("qBr", qBr_l, [128, 2, 768], "k p m -> p k m")
            kB_s = wload("kB", kB_l, [128, 2, 768], "k p m -> p k m")
            kBr_s = wload("kBr", kBr_l, [128, 2, 768], "k p m -> p k m")
            vB_s = wload("vB", vB_l, [128, 2, 768], "k p m -> p k m")
            ow_s = wload("ow", ow_l, [128, 6, 768], "k p m -> p k m")
            cos_s = wp.tile([128, 256], f32, tag="cos", name="cos")
            nc.gpsimd.dma_start(cos_s[:], cos2[:, 0:256])
            sin_s = wp.tile([128, 256], f32, tag="sin", name="sin")
            nc.gpsimd.dma_start(sin_s[:], sin2[:, 0:256])
            mask_s = wp.tile([128, 512], mybir.dt.bfloat16, tag="mask", name="mask")
            nc.gpsimd.dma_start(mask_s[:], mask_bf)
            onec_s = wp.tile([128, 1], f32r, tag="onec", name="onec")
            nc.gpsimd.dma_start(onec_s[:], onec.bitcast(f32r))

            def emit_outproj(aosb_prev, pr_prev, mts, half=None):
                w = 512 if half is None else 256
                c0 = 0 if half in (None, 0) else 256
                tokp = slice(pr_prev * 512 + c0, pr_prev * 512 + c0 + w)
                for mt in mts:
                    fps = psum()
                    for kt in range(6):
                        nc.tensor.matmul(
                            fps[:, 0:w],
                            ow_s[:, kt, mt * 128:(mt + 1) * 128],
                            aosb_prev[:, kt, c0:c0 + w],
                            start=(kt == 0), stop=(kt == 5))
                    fout = fp.tile([128, 512], f32, tag="fout", name="fout")
                    nc.any.tensor_copy(fout[:, 0:w], fps[:, 0:w])
                    nc.sync.dma_start(outT[mt, :, tokp], fout[:, 0:w])

            prev = None
            for pr in range(n_pairs):
                tok = slice(pr * 512, (pr + 1) * 512)
                # ---- load x^T for this pair of batch items ----
                xt = xp.tile([128, 6, 512], f32r, tag="xt", name="xt")
                nc.sync.dma_start(xt[:], xT[:, :, tok].rearrange("k p t -> p k t").bitcast(f32r))

                # ---- proj1: xr^T = A^T @ x^T  (rank padded to 256) ----
                xrs = {}
                for pname, A_s in (("q", qA_s), ("k", kA_s), ("v", vA_s)):
                    mm = [psum(), psum()]
                    for mt in range(2):
                        for kt in range(6):
                            nc.tensor.matmul(
                                mm[mt][:],
                                A_s[:, kt, mt * 64:mt * 64 + 128],
                                xt[:, kt, :],
                                start=(kt == 0), stop=(kt == 5))
                    xr = xrp.tile([128, 2, 512], f32r, tag=f"xr_{pname}", name=f"xr_{pname}")
                    nc.any.tensor_copy(xr[:, 0, :], mm[0][:])
                    nc.any.tensor_copy(xr[:, 1, :], mm[1][:])
                    xrs[pname] = xr

                # ---- proj2 + RoPE for q and k (feature-major) ----
                qsb = qkp.tile([128, 3072], f32r, tag="qsb", name="qsb")
                ksb = qkp.tile([128, 3072], f32r, tag="ksb", name="ksb")
                for g in range(3):
                    for pname, B_s, Br_s, sb in (
                            ("q", qB_s, qBr_s, qsb), ("k", kB_s, kBr_s, ksb)):
                        xr = xrs[pname]
                        tmp = tp.tile([128, 1024], f32, tag="ropetmp",
                                      name="ropetmp", bufs=2)
                        for hh in range(2):
                            h = 2 * g + hh
                            hs = slice(h * 512, (h + 1) * 512)
                            p_main = psumA()
                            p_rot = psumA()
                            for kt in range(2):
                                nc.tensor.matmul(
                                    p_main[:],
                                    B_s[:, kt, h * 128:(h + 1) * 128],
                                    xr[:, kt, :],
                                    start=(kt == 0), stop=(kt == 1))
                            for kt in range(2):
                                nc.tensor.matmul(
                                    p_rot[:],
                                    Br_s[:, kt, h * 128:(h + 1) * 128],
                                    xr[:, kt, :],
                                    start=(kt == 0), stop=(kt == 1))
                            nc.vector.tensor_tensor(
                                sb[:, hs].rearrange("p (b q) -> p b q", b=2),
                                p_main[:].rearrange("p (b q) -> p b q", b=2),
                                cos_s[:, None, :].to_broadcast((128, 2, 256)),
                                mybir.AluOpType.mult)
                            nc.vector.tensor_tensor(
                                tmp[:, hh * 512:(hh + 1) * 512]
                                .rearrange("p (b q) -> p b q", b=2),
                                p_rot[:].rearrange("p (b q) -> p b q", b=2),
                                sin_s[:, None, :].to_broadcast((128, 2, 256)),
                                mybir.AluOpType.mult)
                        gs = slice(2 * g * 512, (2 * g + 2) * 512)
                        nc.vector.tensor_tensor(
                            sb[:, gs], sb[:, gs].bitcast(f32), tmp[:],
                            mybir.AluOpType.add)

                # ---- proj2 for v (token-major) ----
                vsb = qkp.tile([128, 4, 768], f32r, tag="vsb", name="vsb")
                xrv = xrs["v"]
                for mt in range(4):
                    for nch in range(2):
                        vp = psum()
                        for kt in range(2):
                            nc.tensor.matmul(
                                vp[:, 0:384],
                                xrv[:, kt, mt * 128:(mt + 1) * 128],
                                vB_s[:, kt, nch * 384:(nch + 1) * 384],
                                start=(kt == 0), stop=(kt == 1))
                        nc.any.tensor_copy(vsb[:, mt, nch * 384:(nch + 1) * 384],
                                           vp[:, 0:384])

                # ---- attention (per batch item, 2-head groups) ----
                aosb = aop.tile([128, 6, 512], f32r, tag="aosb", name="aosb")
                for b in range(2):
                    if prev is not None:
                        emit_outproj(prev[0], prev[1],
                                     range(3 * b, 3 * b + 3))
                    d_sb = dp.tile([1, 1536], f32, tag="dsb", name="dsb", bufs=1)
                    ddr0 = ddrp.tile([1, 1536], f32, tag="ddr0", name="ddr0")
                    ddr1 = ddrp.tile([1, 1536], f32, tag="ddr1", name="ddr1")
                    oraw = qkp.tile([128, 1536], f32, tag="oraw",
                                    name="oraw", bufs=1)
                    Es = []
                    for g in range(3):
                        E = ep.tile([128, 1024], f32r, tag="E", name="E",
                                    bufs=4)
                        for hh in range(2):
                            h = 2 * g + hh
                            qcol = slice(h * 512 + b * 256,
                                         h * 512 + b * 256 + 256)
                            sp = psum()
                            for kt in range(2):
                                nc.tensor.matmul(
                                    sp[:, kt * 256:(kt + 1) * 256],
                                    ksb[:, h * 512 + b * 256 + kt * 128:
                                        h * 512 + b * 256 + kt * 128 + 128],
                                    qsb[:, qcol],
                                    start=True, stop=True)
                            nc.scalar.activation(
                                E[:, hh * 512:(hh + 1) * 512], sp[:],
                                mybir.ActivationFunctionType.Exp, scale=SCALE)
                        nc.vector.tensor_tensor(
                            E[:].rearrange("p (hh x) -> p hh x", hh=2),
                            E[:].bitcast(f32)
                            .rearrange("p (hh x) -> p hh x", hh=2),
                            mask_s[:, None, :].to_broadcast((128, 2, 512)),
                            mybir.AluOpType.mult)
                        Es.append(E)
                    for g in range(3):
                        E = Es[g]
                        Ev = E[:].rearrange("p (hh kt q) -> p hh kt q",
                                            hh=2, kt=2)
                        dps = psum()
                        for kt in range(2):
                            nc.tensor.matmul(
                                dps[0:1, 0:512], onec_s[:], Ev[:, :, kt, :],
                                start=(kt == 0), stop=(kt == 1))
                        nc.any.tensor_copy(
                            d_sb[0:1, g * 512:(g + 1) * 512], dps[0:1, 0:512])
                        o2 = psum()
                        for hh in range(2):
                            h = 2 * g + hh
                            for kt in range(2):
                                nc.tensor.matmul(
                                    o2[:, hh * 256:(hh + 1) * 256],
                                    vsb[:, b * 2 + kt, h * 128:(h + 1) * 128],
                                    E[:, hh * 512 + kt * 256:
                                      hh * 512 + kt * 256 + 256],
                                    start=(kt == 0), stop=(kt == 1))
                        nc.any.tensor_copy(
                            oraw[:, g * 512:(g + 1) * 512], o2[:])
                    # reciprocal in [128, 12] layout via DRAM reshape
                    nc.sync.dma_start(ddr0[:], d_sb[:])
                    dmat = dp.tile([128, 12], f32, tag="dmat", name="dmat")
                    nc.sync.dma_start(
                        dmat[:], ddr0[0].rearrange("(c p) -> p c", p=128))
                    imat = dp.tile([128, 12], f32, tag="imat", name="imat")
                    nc.vector.reciprocal(imat[:], dmat[:])
                    nc.sync.dma_start(
                        ddr1[0].rearrange("(c p) -> p c", p=128), imat[:])
                    bD = bp.tile([128, 1536], f32, tag="bD", name="bD")
                    nc.sync.dma_start(bD[:], ddr1.to_broadcast((128, 1536)))
                    nc.vector.tensor_tensor(
                        aosb[:, :, b * 256:(b + 1) * 256],
                        oraw[:].rearrange("p (h q) -> p h q", h=6),
                        bD[:].rearrange("p (h q) -> p h q", h=6),
                        mybir.AluOpType.mult)
                prev = (aosb, pr)

            if prev is not None:
                emit_outproj(prev[0], prev[1], range(6), half=0)
                emit_outproj(prev[0], prev[1], range(6), half=1)


    nc.compile()
    return nc


def _rope_tables():
    inv = 1.0 / (10000.0 ** (np.arange(0, HD, 2, dtype=np.float32) / HD))
    t = np.arange(T, dtype=np.float32)
    freqs = np.outer(t, inv)                      # [T, 64]
    emb = np.concatenate([freqs, freqs], axis=-1)  # [T, 128]
    return np.cos(emb).astype(np.float32), np.sin(emb).astype(np.float32)


def _prep_shared(qA, qB, kA, kB, vA, vB, o_w):
    """Host-side weight/constant layouts (shared by all cores)."""
    def a_layout(A):  # [768,192] -> [6,128,192]
        return np.ascontiguousarray(A.reshape(6, 128, RANK))

    def b_layout(Bm):  # [192,768] -> overlapped [2,128,768]
        Bp = np.zeros((2, 128, D), np.float32)
        Bp[0, 0:64] = Bm[0:64]
        Bp[1] = Bm[64:192]
        return np.ascontiguousarray(Bp)

    def rot_weights(Bm):  # fold rotate-half into the projection weights
        Br = np.empty_like(Bm)
        for h in range(H):
            c = h * HD
            Br[:, c:c + 64] = -Bm[:, c + 64:c + 128]
            Br[:, c + 64:c + 128] = Bm[:, c:c + 64]
        return Br

    cos, sin = _rope_tables()
    cosT = np.ascontiguousarray(cos.T)  # [128, 256]
    sinT = np.ascontiguousarray(sin.T)
    cos2 = np.concatenate([cosT, cosT], axis=1)  # [128, 512] (2 batch items)
    sin2 = np.concatenate([sinT, sinT], axis=1)

    p = np.arange(128)[:, None]
    t = np.arange(T)[None, :]
    m0 = (p <= t).astype(np.float32)          # keytile 0
    m1 = (p + 128 <= t).astype(np.float32)    # keytile 1
    mask = np.concatenate([m0, m1], axis=1)   # [128, 512]

    return {
        "qA_l": a_layout(qA), "kA_l": a_layout(kA), "vA_l": a_layout(vA),
        "qB_l": b_layout(qB), "qBr_l": b_layout(rot_weights(qB)),
        "kB_l": b_layout(kB), "kBr_l": b_layout(rot_weights(kB)),
        "vB_l": b_layout(vB),
        "ow_l": np.ascontiguousarray(o_w.reshape(6, 128, D)),
        "cos2": cos2, "sin2": sin2, "mask_bf": _to_bf16(mask),
        "onec": np.ones((128, 1), np.float32),
    }


def x_to_xT(xc):
    """[b, T, D] -> [6, 128, b*T] feature-major, batch-major tokens."""
    nb = xc.shape[0]
    return np.ascontiguousarray(
        xc.reshape(nb, T, 6, 128).transpose(2, 3, 0, 1).reshape(6, 128, nb * T))


def outT_to_out(oT, nb):
    return np.ascontiguousarray(
        oT.reshape(6, 128, nb, T).transpose(2, 3, 0, 1).reshape(nb, T, D))


def kernel(x, qA, qB, kA, kB, vA, vB, o_w):
    from concourse import bass_utils

    if "nc" not in _CACHE:
        _CACHE["nc"] = build_program(N_PAIRS)
    nc = _CACHE["nc"]

    shared = _prep_shared(
        np.asarray(qA, np.float32), np.asarray(qB, np.float32),
        np.asarray(kA, np.float32), np.asarray(kB, np.float32),
        np.asarray(vA, np.float32), np.asarray(vB, np.float32),
        np.asarray(o_w, np.float32))
    x = np.asarray(x, np.float32)

    in_maps = []
    for c in range(N_CORES):
        m = dict(shared)
        m["xT"] = x_to_xT(x[c * B_LOC:(c + 1) * B_LOC])
        in_maps.append(m)

    res = bass_utils.run_bass_kernel_spmd(
        nc, in_maps, core_ids=list(range(N_CORES)))
    out = np.empty((B, T, D), np.float32)
    for c in range(N_CORES):
        out[c * B_LOC:(c + 1) * B_LOC] = outT_to_out(
            res.results[c]["outT"], B_LOC)
    return out



# revision 2
# speedup vs baseline: 1.2968x; 1.2968x over previous
"""Trainium2 Bass kernel for nn_Attn_30623116820602.

Low-rank-projected causal multi-head attention:
  q/k/v = (x @ A) @ B  (rank 192), RoPE on q,k, causal softmax attention,
  output projection.  x: [128, 256, 768] fp32.

Sharding: pure data-parallel over batch (16 items per core, 8 cores).
Device layout is feature-major (d_model on partitions) throughout, so no
on-chip transposes are needed; the host pre-transposes x per core and
post-transposes the result.  RoPE's rotate-half is pre-folded into extra
weight matrices (qB_rot/kB_rot) so the tensor engine emits both q and
rot(q); softmax runs with keys on partitions (no max subtraction --
scores are ~N(0,1)), denominators come from ones-vector matmuls, and the
per-query normalization is broadcast via gpsimd and fused into the
PSUM->SBUF copy of the attention output.

All matmul inputs and SBUF intermediates are bf16 (PSUM accumulation and
the final output stay fp32); the correctness gate is rel_err < 2e-2 and
bf16 keeps us ~50x under it while doubling tensor/vector throughput.
"""

import math
import sys

sys.path.insert(0, "/opt/trn_rl_repo")

import numpy as np
import ml_dtypes


def _to_bf16(a):
    return a.astype(ml_dtypes.bfloat16)


B, T, D = 128, 256, 768
H, HD = 6, 128
RANK = 192  # padded to 256 on host
N_CORES = 8
B_LOC = B // N_CORES  # 16
N_PAIRS = B_LOC // 2  # 8 (2 batch items per pipeline iteration)
SCALE = 1.0 / math.sqrt(HD)

_CACHE = {}


def build_program(n_pairs=N_PAIRS):
    import concourse.tile as tile
    from concourse import bacc, mybir
    from contextlib import ExitStack

    f32 = mybir.dt.float32
    bf16 = mybir.dt.bfloat16
    TOK = n_pairs * 512


    nc = bacc.Bacc("TRN2", target_bir_lowering=False, debug=False,
                   num_devices=N_CORES)

    def din(name, shape):
        return nc.dram_tensor(name, shape, bf16, kind="ExternalInput").ap()

    xT = din("xT", [6, 128, TOK])
    qA_l, kA_l, vA_l = (din(n, [6, 128, 192]) for n in ("qA_l", "kA_l", "vA_l"))
    qB_l, qBr_l, kB_l, kBr_l, vB_l = (
        din(n, [2, 128, 768]) for n in ("qB_l", "qBr_l", "kB_l", "kBr_l", "vB_l"))
    ow_l = din("ow_l", [6, 128, 768])
    cos2 = din("cos2", [128, 512])
    sin2 = din("sin2", [128, 512])
    mask_bf = din("mask_bf", [128, 512])
    onec = din("onec", [128, 1])
    outT = nc.dram_tensor("outT", [6, 128, TOK], f32, kind="ExternalOutput").ap()

    with tile.TileContext(nc) as tc:
        with ExitStack() as ctx:
            wp = ctx.enter_context(tc.tile_pool(name="w", bufs=1))
            xp = ctx.enter_context(tc.tile_pool(name="xt", bufs=2))
            xrp = ctx.enter_context(tc.tile_pool(name="xr", bufs=1))
            qkp = ctx.enter_context(tc.tile_pool(name="qk", bufs=1))
            tp = ctx.enter_context(tc.tile_pool(name="tmp", bufs=2))
            ep = ctx.enter_context(tc.tile_pool(name="eexp", bufs=3))
            dp = ctx.enter_context(tc.tile_pool(name="den", bufs=2))
            bp = ctx.enter_context(tc.tile_pool(name="bcast", bufs=1))
            aop = ctx.enter_context(tc.tile_pool(name="ao", bufs=2))
            fp = ctx.enter_context(tc.tile_pool(name="fout", bufs=1))
            ddrp = ctx.enter_context(tc.tile_pool(name="ddr", bufs=2, space="DRAM"))
            ps = ctx.enter_context(tc.tile_pool(name="ps", bufs=5, space="PSUM"))
            psa = ctx.enter_context(tc.tile_pool(name="psa", bufs=3, space="PSUM"))

            def psum():
                return ps.tile([128, 512], f32, tag="ps", name="psb")

            def psumA():
                return psa.tile([128, 512], f32, tag="psa", name="psa")

            # ---- resident weights / constants ----
            def wload(name, src, shape, perm):
                t = wp.tile(shape, bf16, tag=name, name=name)
                nc.gpsimd.dma_start(t[:], src.rearrange(perm))
                return t

            qA_s = wload("qA", qA_l, [128, 6, 192], "k p m -> p k m")
            kA_s = wload("kA", kA_l, [128, 6, 192], "k p m -> p k m")
            vA_s = wload("vA", vA_l, [128, 6, 192], "k p m -> p k m")
            qB_s = wload("qB", qB_l, [128, 2, 768], "k p m -> p k m")
            qBr_s = wload("qBr", qBr_l, [128, 2, 768], "k p m -> p k m")
            kB_s = wload("kB", kB_l, [128, 2, 768], "k p m -> p k m")
            kBr_s = wload("kBr", kBr_l, [128, 2, 768], "k p m -> p k m")
            vB_s = wload("vB", vB_l, [128, 2, 768], "k p m -> p k m")
            ow_s = wload("ow", ow_l, [128, 6, 768], "k p m -> p k m")
            cos_s = wp.tile([128, 256], bf16, tag="cos", name="cos")
            nc.gpsimd.dma_start(cos_s[:], cos2[:, 0:256])
            sin_s = wp.tile([128, 256], bf16, tag="sin", name="sin")
            nc.gpsimd.dma_start(sin_s[:], sin2[:, 0:256])
            mask_s = wp.tile([128, 512], bf16, tag="mask", name="mask")
            nc.gpsimd.dma_start(mask_s[:], mask_bf)
            onec_s = wp.tile([128, 1], bf16, tag="onec", name="onec")
            nc.gpsimd.dma_start(onec_s[:], onec)

            def emit_outproj(aosb_prev, pr_prev, mts, half=None):
                w = 512 if half is None else 256
                c0 = 0 if half in (None, 0) else 256
                tokp = slice(pr_prev * 512 + c0, pr_prev * 512 + c0 + w)
                for mt in mts:
                    fps = psum()
                    for kt in range(6):
                        nc.tensor.matmul(
                            fps[:, 0:w],
                            ow_s[:, kt, mt * 128:(mt + 1) * 128],
                            aosb_prev[:, kt, c0:c0 + w],
                            start=(kt == 0), stop=(kt == 5))
                    fout = fp.tile([128, 512], f32, tag="fout", name="fout")
                    nc.any.tensor_copy(fout[:, 0:w], fps[:, 0:w])
                    nc.sync.dma_start(outT[mt, :, tokp], fout[:, 0:w])

            prev = None
            for pr in range(n_pairs):
                tok = slice(pr * 512, (pr + 1) * 512)
                # ---- load x^T for this pair of batch items ----
                xt = xp.tile([128, 6, 512], bf16, tag="xt", name="xt")
                nc.sync.dma_start(xt[:], xT[:, :, tok].rearrange("k p t -> p k t"))

                # ---- proj1: xr^T = A^T @ x^T  (rank padded to 256) ----
                xrs = {}
                for pname, A_s in (("q", qA_s), ("k", kA_s), ("v", vA_s)):
                    mm = [psum(), psum()]
                    for mt in range(2):
                        for kt in range(6):
                            nc.tensor.matmul(
                                mm[mt][:],
                                A_s[:, kt, mt * 64:mt * 64 + 128],
                                xt[:, kt, :],
                                start=(kt == 0), stop=(kt == 5))
                    xr = xrp.tile([128, 2, 512], bf16, tag=f"xr_{pname}", name=f"xr_{pname}")
                    nc.any.tensor_copy(xr[:, 0, :], mm[0][:])
                    nc.any.tensor_copy(xr[:, 1, :], mm[1][:])
                    xrs[pname] = xr

                # ---- proj2 + RoPE for q and k (feature-major) ----
                qsb = qkp.tile([128, 3072], bf16, tag="qsb", name="qsb")
                ksb = qkp.tile([128, 3072], bf16, tag="ksb", name="ksb")
                for g in range(3):
                    for pname, B_s, Br_s, sb in (
                            ("q", qB_s, qBr_s, qsb), ("k", kB_s, kBr_s, ksb)):
                        xr = xrs[pname]
                        tmp = tp.tile([128, 1024], bf16, tag="ropetmp",
                                      name="ropetmp", bufs=2)
                        for hh in range(2):
                            h = 2 * g + hh
                            hs = slice(h * 512, (h + 1) * 512)
                            p_main = psumA()
                            p_rot = psumA()
                            for kt in range(2):
                                nc.tensor.matmul(
                                    p_main[:],
                                    B_s[:, kt, h * 128:(h + 1) * 128],
                                    xr[:, kt, :],
                                    start=(kt == 0), stop=(kt == 1))
                            for kt in range(2):
                                nc.tensor.matmul(
                                    p_rot[:],
                                    Br_s[:, kt, h * 128:(h + 1) * 128],
                                    xr[:, kt, :],
                                    start=(kt == 0), stop=(kt == 1))
                            nc.vector.tensor_tensor(
                                sb[:, hs].rearrange("p (b q) -> p b q", b=2),
                                p_main[:].rearrange("p (b q) -> p b q", b=2),
                                cos_s[:, None, :].to_broadcast((128, 2, 256)),
                                mybir.AluOpType.mult)
                            nc.vector.tensor_tensor(
                                tmp[:, hh * 512:(hh + 1) * 512]
                                .rearrange("p (b q) -> p b q", b=2),
                                p_rot[:].rearrange("p (b q) -> p b q", b=2),
                                sin_s[:, None, :].to_broadcast((128, 2, 256)),
                                mybir.AluOpType.mult)
                        gs = slice(2 * g * 512, (2 * g + 2) * 512)
                        nc.vector.tensor_tensor(
                            sb[:, gs], sb[:, gs], tmp[:],
                            mybir.AluOpType.add)

                # ---- proj2 for v (token-major) ----
                vsb = qkp.tile([128, 4, 768], bf16, tag="vsb", name="vsb")
                xrv = xrs["v"]
                for mt in range(4):
                    for nch in range(2):
                        vp = psum()
                        for kt in range(2):
                            nc.tensor.matmul(
                                vp[:, 0:384],
                                xrv[:, kt, mt * 128:(mt + 1) * 128],
                                vB_s[:, kt, nch * 384:(nch + 1) * 384],
                                start=(kt == 0), stop=(kt == 1))
                        nc.any.tensor_copy(vsb[:, mt, nch * 384:(nch + 1) * 384],
                                           vp[:, 0:384])

                # ---- attention (per batch item, 2-head groups) ----
                aosb = aop.tile([128, 6, 512], bf16, tag="aosb", name="aosb")
                for b in range(2):
                    if prev is not None:
                        emit_outproj(prev[0], prev[1],
                                     range(3 * b, 3 * b + 3))
                    d_sb = dp.tile([1, 1536], f32, tag="dsb", name="dsb", bufs=1)
                    ddr0 = ddrp.tile([1, 1536], f32, tag="ddr0", name="ddr0")
                    ddr1 = ddrp.tile([1, 1536], f32, tag="ddr1", name="ddr1")
                    oraw = qkp.tile([128, 1536], bf16, tag="oraw",
                                    name="oraw", bufs=1)
                    Es = []
                    for g in range(3):
                        E = ep.tile([128, 1024], bf16, tag="E", name="E",
                                    bufs=4)
                        for hh in range(2):
                            h = 2 * g + hh
                            qcol = slice(h * 512 + b * 256,
                                         h * 512 + b * 256 + 256)
                            sp = psum()
                            for kt in range(2):
                                nc.tensor.matmul(
                                    sp[:, kt * 256:(kt + 1) * 256],
                                    ksb[:, h * 512 + b * 256 + kt * 128:
                                        h * 512 + b * 256 + kt * 128 + 128],
                                    qsb[:, qcol],
                                    start=True, stop=True)
                            nc.scalar.activation(
                                E[:, hh * 512:(hh + 1) * 512], sp[:],
                                mybir.ActivationFunctionType.Exp, scale=SCALE)
                        nc.vector.tensor_tensor(
                            E[:].rearrange("p (hh x) -> p hh x", hh=2),
                            E[:].rearrange("p (hh x) -> p hh x", hh=2),
                            mask_s[:, None, :].to_broadcast((128, 2, 512)),
                            mybir.AluOpType.mult)
                        Es.append(E)
                    for g in range(3):
                        E = Es[g]
                        Ev = E[:].rearrange("p (hh kt q) -> p hh kt q",
                                            hh=2, kt=2)
                        dps = psum()
                        for kt in range(2):
                            nc.tensor.matmul(
                                dps[0:1, 0:512], onec_s[:], Ev[:, :, kt, :],
                                start=(kt == 0), stop=(kt == 1))
                        nc.any.tensor_copy(
                            d_sb[0:1, g * 512:(g + 1) * 512], dps[0:1, 0:512])
                        o2 = psum()
                        for hh in range(2):
                            h = 2 * g + hh
                            for kt in range(2):
                                nc.tensor.matmul(
                                    o2[:, hh * 256:(hh + 1) * 256],
                                    vsb[:, b * 2 + kt, h * 128:(h + 1) * 128],
                                    E[:, hh * 512 + kt * 256:
                                      hh * 512 + kt * 256 + 256],
                                    start=(kt == 0), stop=(kt == 1))
                        nc.any.tensor_copy(
                            oraw[:, g * 512:(g + 1) * 512], o2[:])
                    # reciprocal in [128, 12] layout via DRAM reshape
                    nc.sync.dma_start(ddr0[:], d_sb[:])
                    dmat = dp.tile([128, 12], f32, tag="dmat", name="dmat")
                    nc.sync.dma_start(
                        dmat[:], ddr0[0].rearrange("(c p) -> p c", p=128))
                    imat = dp.tile([128, 12], f32, tag="imat", name="imat")
                    nc.vector.reciprocal(imat[:], dmat[:])
                    nc.sync.dma_start(
                        ddr1[0].rearrange("(c p) -> p c", p=128), imat[:])
                    bD = bp.tile([128, 1536], f32, tag="bD", name="bD")
                    nc.sync.dma_start(bD[:], ddr1.to_broadcast((128, 1536)))
                    nc.vector.tensor_tensor(
                        aosb[:, :, b * 256:(b + 1) * 256],
                        oraw[:].rearrange("p (h q) -> p h q", h=6),
                        bD[:].rearrange("p (h q) -> p h q", h=6),
                        mybir.AluOpType.mult)
                prev = (aosb, pr)

            if prev is not None:
                emit_outproj(prev[0], prev[1], range(6), half=0)
                emit_outproj(prev[0], prev[1], range(6), half=1)


    nc.compile()
    return nc


def _rope_tables():
    inv = 1.0 / (10000.0 ** (np.arange(0, HD, 2, dtype=np.float32) / HD))
    t = np.arange(T, dtype=np.float32)
    freqs = np.outer(t, inv)                      # [T, 64]
    emb = np.concatenate([freqs, freqs], axis=-1)  # [T, 128]
    return np.cos(emb).astype(np.float32), np.sin(emb).astype(np.float32)


def _prep_shared(qA, qB, kA, kB, vA, vB, o_w):
    """Host-side weight/constant layouts (shared by all cores)."""
    def a_layout(A):  # [768,192] -> [6,128,192]
        return np.ascontiguousarray(A.reshape(6, 128, RANK))

    def b_layout(Bm):  # [192,768] -> overlapped [2,128,768]
        Bp = np.zeros((2, 128, D), np.float32)
        Bp[0, 0:64] = Bm[0:64]
        Bp[1] = Bm[64:192]
        return np.ascontiguousarray(Bp)

    def rot_weights(Bm):  # fold rotate-half into the projection weights
        Br = np.empty_like(Bm)
        for h in range(H):
            c = h * HD
            Br[:, c:c + 64] = -Bm[:, c + 64:c + 128]
            Br[:, c + 64:c + 128] = Bm[:, c:c + 64]
        return Br

    cos, sin = _rope_tables()
    cosT = np.ascontiguousarray(cos.T)  # [128, 256]
    sinT = np.ascontiguousarray(sin.T)
    cos2 = np.concatenate([cosT, cosT], axis=1)  # [128, 512] (2 batch items)
    sin2 = np.concatenate([sinT, sinT], axis=1)

    p = np.arange(128)[:, None]
    t = np.arange(T)[None, :]
    m0 = (p <= t).astype(np.float32)          # keytile 0
    m1 = (p + 128 <= t).astype(np.float32)    # keytile 1
    mask = np.concatenate([m0, m1], axis=1)   # [128, 512]

    return {
        "qA_l": _to_bf16(a_layout(qA)), "kA_l": _to_bf16(a_layout(kA)),
        "vA_l": _to_bf16(a_layout(vA)),
        "qB_l": _to_bf16(b_layout(qB)), "qBr_l": _to_bf16(b_layout(rot_weights(qB))),
        "kB_l": _to_bf16(b_layout(kB)), "kBr_l": _to_bf16(b_layout(rot_weights(kB))),
        "vB_l": _to_bf16(b_layout(vB)),
        "ow_l": _to_bf16(np.ascontiguousarray(o_w.reshape(6, 128, D))),
        "cos2": _to_bf16(cos2), "sin2": _to_bf16(sin2), "mask_bf": _to_bf16(mask),
        "onec": _to_bf16(np.ones((128, 1), np.float32)),
    }


def x_to_xT(xc):
    """[b, T, D] -> [6, 128, b*T] feature-major, batch-major tokens."""
    nb = xc.shape[0]
    return np.ascontiguousarray(
        _to_bf16(xc).reshape(nb, T, 6, 128).transpose(2, 3, 0, 1).reshape(6, 128, nb * T))


def outT_to_out(oT, nb):
    return np.ascontiguousarray(
        oT.reshape(6, 128, nb, T).transpose(2, 3, 0, 1).reshape(nb, T, D))


def kernel(x, qA, qB, kA, kB, vA, vB, o_w):
    from concourse import bass_utils

    if "nc" not in _CACHE:
        _CACHE["nc"] = build_program(N_PAIRS)
    nc = _CACHE["nc"]

    shared = _prep_shared(
        np.asarray(qA, np.float32), np.asarray(qB, np.float32),
        np.asarray(kA, np.float32), np.asarray(kB, np.float32),
        np.asarray(vA, np.float32), np.asarray(vB, np.float32),
        np.asarray(o_w, np.float32))
    x = np.asarray(x, np.float32)

    in_maps = []
    for c in range(N_CORES):
        m = dict(shared)
        m["xT"] = x_to_xT(x[c * B_LOC:(c + 1) * B_LOC])
        in_maps.append(m)

    res = bass_utils.run_bass_kernel_spmd(
        nc, in_maps, core_ids=list(range(N_CORES)))
    out = np.empty((B, T, D), np.float32)
    for c in range(N_CORES):
        out[c * B_LOC:(c + 1) * B_LOC] = outT_to_out(
            res.results[c]["outT"], B_LOC)
    return out


# revision 3
# speedup vs baseline: 1.4901x; 1.1491x over previous
"""Trainium2 Bass kernel for nn_Attn_30623116820602.

Low-rank-projected causal multi-head attention:
  q/k/v = (x @ A) @ B  (rank 192), RoPE on q,k, causal softmax attention,
  output projection.  x: [128, 256, 768] fp32.

Sharding: pure data-parallel over batch (16 items per core, 8 cores).
Feature-major layout (d_model on partitions) throughout; host pre/post
transposes.  All matmul inputs are bf16 (PSUM accumulates fp32).

Structure (per pair of batch items = 512 token columns):
  - proj1 packs the 3 rank-192 outputs into 5 (not 6) 128-row tiles:
    [q0:128 | q128:192+k0:64 | k64:192 | v0:128 | v128:192+pad].
  - RoPE rotate-half comes from one extra matmul with a shared 128x128
    +-1 permutation matrix P (contraction 128) instead of duplicated
    rank-contraction weight matmuls.
  - Causal block structure is exploited: the fully-masked
    (keytile1 x querytile0) block is never computed -- not in scores,
    exp, denominators, nor the AV matmul.  E layout per (item, head) is
    [kt0q0 | kt1q1 | kt0q1] so the two triangular diagonal blocks are
    adjacent and share one mask multiply.
  - Softmax denominators: ones-vector matmuls -> PSUM, reciprocal on
    vector engine straight out of PSUM, partition-broadcast on the idle
    gpsimd engine.  No DRAM round trip (the fp32 baseline's 4-hop DRAM
    chain serialized the pipeline at ~16us per pair).
  - x loads are prefetched one pair ahead on the gpsimd DMA queue;
    output stores ride the sync queue; the output projection of pair
    N-1 is interleaved into pair N's attention to keep the PE busy.
"""

import math
import sys

sys.path.insert(0, "/opt/trn_rl_repo")

import numpy as np
import ml_dtypes


def _to_bf16(a):
    return a.astype(ml_dtypes.bfloat16)


B, T, D = 128, 256, 768
H, HD = 6, 128
RANK = 192
N_CORES = 8
B_LOC = B // N_CORES  # 16
N_PAIRS = B_LOC // 2  # 8 (2 batch items per pipeline iteration)
SCALE = 1.0 / math.sqrt(HD)

_CACHE = {}


def build_program(n_pairs=N_PAIRS):
    import concourse.tile as tile
    from concourse import bacc, mybir
    from contextlib import ExitStack

    f32 = mybir.dt.float32
    bf16 = mybir.dt.bfloat16
    TOK = n_pairs * 512

    nc = bacc.Bacc("TRN2", target_bir_lowering=False, debug=False,
                   num_devices=N_CORES)

    def din(name, shape):
        return nc.dram_tensor(name, shape, bf16, kind="ExternalInput").ap()

    xT = din("xT", [6, 128, TOK])
    Ap_l = din("Ap_l", [6, 128, 640])
    qBp_l = din("qBp_l", [2, 128, 768])
    kBp_l = din("kBp_l", [2, 128, 768])
    vBp_l = din("vBp_l", [2, 128, 768])
    ow_l = din("ow_l", [6, 128, 768])
    P_l = din("P_l", [128, 128])
    cosT = din("cosT", [128, 256])
    sinT = din("sinT", [128, 256])
    tril_l = din("tril_l", [128, 128])
    onec = din("onec", [128, 1])
    outT = nc.dram_tensor("outT", [6, 128, TOK], f32, kind="ExternalOutput").ap()

    with tile.TileContext(nc) as tc:
        with ExitStack() as ctx:
            wp = ctx.enter_context(tc.tile_pool(name="w", bufs=1))
            xp = ctx.enter_context(tc.tile_pool(name="xt", bufs=2))
            xrp = ctx.enter_context(tc.tile_pool(name="xr", bufs=2))
            rawp = ctx.enter_context(tc.tile_pool(name="raw", bufs=2))
            qkp = ctx.enter_context(tc.tile_pool(name="qk", bufs=1))
            vp_ = ctx.enter_context(tc.tile_pool(name="vsb", bufs=2))
            tp = ctx.enter_context(tc.tile_pool(name="tmp", bufs=2))
            ep = ctx.enter_context(tc.tile_pool(name="eexp", bufs=8))
            dp = ctx.enter_context(tc.tile_pool(name="den", bufs=2))
            bp = ctx.enter_context(tc.tile_pool(name="bcast", bufs=2))
            orp = ctx.enter_context(tc.tile_pool(name="oraw", bufs=2))
            aop = ctx.enter_context(tc.tile_pool(name="ao", bufs=2))
            fp = ctx.enter_context(tc.tile_pool(name="fout", bufs=2))
            ps = ctx.enter_context(tc.tile_pool(name="ps", bufs=3, space="PSUM"))
            pm = ctx.enter_context(tc.tile_pool(name="pm", bufs=2, space="PSUM"))
            pr = ctx.enter_context(tc.tile_pool(name="pr", bufs=1, space="PSUM"))
            spp = ctx.enter_context(tc.tile_pool(name="sp", bufs=2, space="PSUM"))

            def psum():
                return ps.tile([128, 512], f32, tag="ps", name="psb")

            # ---- resident weights / constants (gpsimd DMA queue) ----
            def wload(name, src, shape, perm=None):
                t = wp.tile(shape, bf16, tag=name, name=name)
                nc.gpsimd.dma_start(t[:], src.rearrange(perm) if perm else src)
                return t

            A_s = wload("Ap", Ap_l, [128, 6, 640], "k p m -> p k m")
            qBp_s = wload("qBp", qBp_l, [128, 2, 768], "k p m -> p k m")
            kBp_s = wload("kBp", kBp_l, [128, 2, 768], "k p m -> p k m")
            vBp_s = wload("vBp", vBp_l, [128, 2, 768], "k p m -> p k m")
            ow_s = wload("ow", ow_l, [128, 6, 768], "k p m -> p k m")
            P_s = wload("P", P_l, [128, 128])
            cos_s = wload("cos", cosT, [128, 256])
            sin_s = wload("sin", sinT, [128, 256])
            tril_s = wload("tril", tril_l, [128, 128])
            onec_s = wload("onec", onec, [128, 1])

            def emit_outproj(aosb_prev, pr_prev, mts, half=None):
                w = 512 if half is None else 256
                c0 = 0 if half in (None, 0) else 256
                tokp = slice(pr_prev * 512 + c0, pr_prev * 512 + c0 + w)
                for mt in mts:
                    fps = psum()
                    for kt in range(6):
                        nc.tensor.matmul(
                            fps[:, 0:w],
                            ow_s[:, kt, mt * 128:(mt + 1) * 128],
                            aosb_prev[:, kt, c0:c0 + w],
                            start=(kt == 0), stop=(kt == 5))
                    fout = fp.tile([128, 512], f32, tag="fout", name="fout")
                    nc.scalar.copy(fout[:, 0:w], fps[:, 0:w])
                    nc.sync.dma_start(outT[mt, :, tokp], fout[:, 0:w])

            # prefetch first x pair
            xts = [None] * n_pairs

            def load_xt(p):
                t = xp.tile([128, 6, 512], bf16, tag="xt", name="xt")
                nc.gpsimd.dma_start(
                    t[:], xT[:, :, p * 512:(p + 1) * 512].rearrange("k p t -> p k t"))
                xts[p] = t

            load_xt(0)

            prev = None
            for prx in range(n_pairs):
                if prx + 1 < n_pairs:
                    load_xt(prx + 1)
                xt = xts[prx]

                # ---- proj1: packed rank tiles [q|q+k|k|v|v] ----
                xr = xrp.tile([128, 5, 512], bf16, tag="xr", name="xr")
                for rt in range(5):
                    mm = psum()
                    for kt in range(6):
                        nc.tensor.matmul(
                            mm[:],
                            A_s[:, kt, rt * 128:(rt + 1) * 128],
                            xt[:, kt, :],
                            start=(kt == 0), stop=(kt == 5))
                    nc.scalar.copy(xr[:, rt, :], mm[:])

                # ---- proj2 + RoPE for q and k (feature-major) ----
                # q contracts xr tiles {0,1}; k contracts {1,2} (B rows
                # zero-padded on host where tiles are shared).
                qsb = qkp.tile([128, 6, 512], bf16, tag="qsb", name="qsb")
                ksb = qkp.tile([128, 6, 512], bf16, tag="ksb", name="ksb")
                for h in range(6):
                    hc = slice(h * 128, (h + 1) * 128)
                    mains = {}
                    for pname, B_s, t0, sb in (
                            ("q", qBp_s, 0, qsb), ("k", kBp_s, 1, ksb)):
                        p_main = pm.tile([128, 512], f32, tag="pm", name="pm")
                        for kt in range(2):
                            nc.tensor.matmul(
                                p_main[:], B_s[:, kt, hc], xr[:, t0 + kt, :],
                                start=(kt == 0), stop=(kt == 1))
                        raw = rawp.tile([128, 512], bf16, tag=f"raw{pname}",
                                        name=f"raw{pname}")
                        nc.scalar.copy(raw[:], p_main[:])
                        mains[pname] = (p_main, raw, sb)
                    for pname in ("q", "k"):
                        p_main, raw, sb = mains[pname]
                        p_rot = pr.tile([128, 512], f32, tag="pr", name="pr")
                        nc.tensor.matmul(p_rot[:], P_s[:], raw[:],
                                         start=True, stop=True)
                        tmp = tp.tile([128, 512], bf16, tag="ropetmp",
                                      name="ropetmp")
                        nc.vector.tensor_tensor(
                            sb[:, h, :].rearrange("p (b q) -> p b q", b=2),
                            p_main[:].rearrange("p (b q) -> p b q", b=2),
                            cos_s[:, None, :].to_broadcast((128, 2, 256)),
                            mybir.AluOpType.mult)
                        nc.vector.tensor_tensor(
                            tmp[:].rearrange("p (b q) -> p b q", b=2),
                            p_rot[:].rearrange("p (b q) -> p b q", b=2),
                            sin_s[:, None, :].to_broadcast((128, 2, 256)),
                            mybir.AluOpType.mult)
                        nc.vector.tensor_tensor(
                            sb[:, h, :], sb[:, h, :], tmp[:],
                            mybir.AluOpType.add)

                # ---- proj2 for v (token-major), contracts xr tiles {3,4} ----
                vsb = vp_.tile([128, 4, 768], bf16, tag="vsb", name="vsb")
                for mt in range(4):
                    for nch in range(2):
                        vps = psum()
                        for kt in range(2):
                            nc.tensor.matmul(
                                vps[:, 0:384],
                                xr[:, 3 + kt, mt * 128:(mt + 1) * 128],
                                vBp_s[:, kt, nch * 384:(nch + 1) * 384],
                                start=(kt == 0), stop=(kt == 1))
                        nc.scalar.copy(vsb[:, mt, nch * 384:(nch + 1) * 384],
                                       vps[:, 0:384])

                # ---- attention (per batch item) ----
                # E layout per (b, h): [kt0q0 | kt1q1 | kt0q1], each 128 wide.
                aosb = aop.tile([128, 6, 512], bf16, tag="aosb", name="aosb")
                for b in range(2):
                    if prev is not None:
                        emit_outproj(prev[0], prev[1], range(3 * b, 3 * b + 3))
                    oraw = orp.tile([128, 1536], bf16, tag="oraw", name="oraw")
                    i_sb = dp.tile([1, 1536], f32, tag="isb", name="isb")
                    bD = bp.tile([128, 1536], f32, tag="bD", name="bD")
                    Es = []
                    for h in range(6):
                        base = h * 512 + 0  # qsb is [128, 6, 512]
                        qh = qsb[:, h, b * 256:(b + 1) * 256]
                        kh = ksb[:, h, b * 256:(b + 1) * 256]
                        sp = spp.tile([128, 512], f32, tag="sp", name="sp")
                        # kt0 stationary shared by the two N=128 matmuls
                        nc.tensor.matmul(sp[:, 0:128], kh[:, 0:128],
                                         qh[:, 0:128], start=True, stop=True)
                        nc.tensor.matmul(sp[:, 256:384], kh[:, 0:128],
                                         qh[:, 128:256], start=True, stop=True)
                        nc.tensor.matmul(sp[:, 128:256], kh[:, 128:256],
                                         qh[:, 128:256], start=True, stop=True)
                        E = ep.tile([128, 384], bf16, tag="E", name="E")
                        nc.scalar.activation(
                            E[:], sp[:, 0:384],
                            mybir.ActivationFunctionType.Exp, scale=SCALE)
                        # mask the two adjacent triangular diagonal blocks
                        nc.vector.tensor_tensor(
                            E[:, 0:256].rearrange("p (x q) -> p x q", x=2),
                            E[:, 0:256].rearrange("p (x q) -> p x q", x=2),
                            tril_s[:, None, :].to_broadcast((128, 2, 128)),
                            mybir.AluOpType.mult)
                        Es.append(E)
                    # denominators: 18 ones-matmuls sharing one stationary
                    dslots = []
                    for h in range(6):
                        E = Es[h]
                        s, c = h // 2, (h % 2) * 256
                        if h % 2 == 0:
                            dslots.append(psum())
                        dps = dslots[s]
                        nc.tensor.matmul(dps[0:1, c:c + 128], onec_s[:],
                                         E[:, 0:128], start=True, stop=True)
                        nc.tensor.matmul(dps[0:1, c + 128:c + 256], onec_s[:],
                                         E[:, 128:256], start=True, stop=False)
                        nc.tensor.matmul(dps[0:1, c + 128:c + 256], onec_s[:],
                                         E[:, 256:384], start=False, stop=True)
                        if h % 2 == 1:
                            sc = slice(s * 512, (s + 1) * 512)
                            nc.vector.reciprocal(i_sb[0:1, sc], dps[0:1, :])
                            nc.gpsimd.partition_broadcast(
                                bD[:, sc], i_sb[0:1, sc])
                    # attention output (feature-major), per head
                    for h in range(6):
                        E = Es[h]
                        o2 = psum()
                        v0 = vsb[:, b * 2 + 0, h * 128:(h + 1) * 128]
                        v1 = vsb[:, b * 2 + 1, h * 128:(h + 1) * 128]
                        nc.tensor.matmul(o2[:, 0:128], v0, E[:, 0:128],
                                         start=True, stop=True)
                        nc.tensor.matmul(o2[:, 128:256], v0, E[:, 256:384],
                                         start=True, stop=False)
                        nc.tensor.matmul(o2[:, 128:256], v1, E[:, 128:256],
                                         start=False, stop=True)
                        nc.scalar.copy(oraw[:, h * 256:(h + 1) * 256],
                                       o2[:, 0:256])
                    # normalize into aosb (outproj layout)
                    for s in range(3):
                        sc = slice(s * 512, (s + 1) * 512)
                        nc.vector.tensor_tensor(
                            aosb[:, 2 * s:2 * s + 2, b * 256:(b + 1) * 256],
                            oraw[:, sc].rearrange("p (h q) -> p h q", h=2),
                            bD[:, sc].rearrange("p (h q) -> p h q", h=2),
                            mybir.AluOpType.mult)
                prev = (aosb, prx)

            if prev is not None:
                emit_outproj(prev[0], prev[1], range(6), half=0)
                emit_outproj(prev[0], prev[1], range(6), half=1)

    nc.compile()
    return nc


def _rope_tables():
    inv = 1.0 / (10000.0 ** (np.arange(0, HD, 2, dtype=np.float32) / HD))
    t = np.arange(T, dtype=np.float32)
    freqs = np.outer(t, inv)                      # [T, 64]
    emb = np.concatenate([freqs, freqs], axis=-1)  # [T, 128]
    return np.cos(emb).astype(np.float32), np.sin(emb).astype(np.float32)


def _prep_shared(qA, qB, kA, kB, vA, vB, o_w):
    """Host-side weight/constant layouts (shared by all cores)."""
    def a_r(A):  # [768,192] -> [6,128,192]
        return A.reshape(6, 128, RANK)

    qA_r, kA_r, vA_r = a_r(qA), a_r(kA), a_r(vA)
    Ap = np.zeros((6, 128, 640), np.float32)
    Ap[:, :, 0:128] = qA_r[:, :, 0:128]
    Ap[:, :, 128:192] = qA_r[:, :, 128:192]
    Ap[:, :, 192:256] = kA_r[:, :, 0:64]
    Ap[:, :, 256:384] = kA_r[:, :, 64:192]
    Ap[:, :, 384:512] = vA_r[:, :, 0:128]
    Ap[:, :, 512:576] = vA_r[:, :, 128:192]

    qBp = np.zeros((2, 128, D), np.float32)
    qBp[0] = qB[0:128]
    qBp[1, 0:64] = qB[128:192]

    kBp = np.zeros((2, 128, D), np.float32)
    kBp[0, 64:128] = kB[0:64]
    kBp[1] = kB[64:192]

    vBp = np.zeros((2, 128, D), np.float32)
    vBp[0] = vB[0:128]
    vBp[1, 0:64] = vB[128:192]

    # rotate-half permutation (as matmul lhsT): out[m] = sum_k P[k,m] q[k]
    P = np.zeros((128, 128), np.float32)
    for m in range(64):
        P[m + 64, m] = -1.0
        P[m, m + 64] = 1.0

    cos, sin = _rope_tables()
    cosT = np.ascontiguousarray(cos.T)  # [128, 256]
    sinT = np.ascontiguousarray(sin.T)

    p = np.arange(128)[:, None]
    j = np.arange(128)[None, :]
    tril = (p <= j).astype(np.float32)  # keys (partitions) <= queries (cols)

    return {
        "Ap_l": _to_bf16(np.ascontiguousarray(Ap)),
        "qBp_l": _to_bf16(qBp), "kBp_l": _to_bf16(kBp), "vBp_l": _to_bf16(vBp),
        "ow_l": _to_bf16(np.ascontiguousarray(o_w.reshape(6, 128, D))),
        "P_l": _to_bf16(P),
        "cosT": _to_bf16(cosT), "sinT": _to_bf16(sinT),
        "tril_l": _to_bf16(tril),
        "onec": _to_bf16(np.ones((128, 1), np.float32)),
    }


def x_to_xT(xc):
    """[b, T, D] -> [6, 128, b*T] feature-major, batch-major tokens."""
    nb = xc.shape[0]
    return np.ascontiguousarray(
        _to_bf16(xc).reshape(nb, T, 6, 128).transpose(2, 3, 0, 1).reshape(6, 128, nb * T))


def outT_to_out(oT, nb):
    return np.ascontiguousarray(
        oT.reshape(6, 128, nb, T).transpose(2, 3, 0, 1).reshape(nb, T, D))


def kernel(x, qA, qB, kA, kB, vA, vB, o_w):
    from concourse import bass_utils

    if "nc" not in _CACHE:
        _CACHE["nc"] = build_program(N_PAIRS)
    nc = _CACHE["nc"]

    shared = _prep_shared(
        np.asarray(qA, np.float32), np.asarray(qB, np.float32),
        np.asarray(kA, np.float32), np.asarray(kB, np.float32),
        np.asarray(vA, np.float32), np.asarray(vB, np.float32),
        np.asarray(o_w, np.float32))
    x = np.asarray(x, np.float32)

    in_maps = []
    for c in range(N_CORES):
        m = dict(shared)
        m["xT"] = x_to_xT(x[c * B_LOC:(c + 1) * B_LOC])
        in_maps.append(m)

    res = bass_utils.run_bass_kernel_spmd(
        nc, in_maps, core_ids=list(range(N_CORES)))
    out = np.empty((B, T, D), np.float32)
    for c in range(N_CORES):
        out[c * B_LOC:(c + 1) * B_LOC] = outT_to_out(
            res.results[c]["outT"], B_LOC)
    return out


# revision 10
# speedup vs baseline: 1.5772x; 1.0585x over previous
"""Trainium2 Bass kernel for nn_Attn_30623116820602.

Low-rank-projected causal multi-head attention:
  q/k/v = (x @ A) @ B  (rank 192), RoPE on q,k, causal softmax attention,
  output projection.  x: [128, 256, 768] fp32.

Sharding: pure data-parallel over batch (16 items per core, 8 cores).
Feature-major layout (d_model on partitions) throughout; host pre/post
transposes.  All matmul inputs are bf16 (PSUM accumulates fp32).

Structure (per pair of batch items = 512 token columns):
  - proj1 packs the 3 rank-192 outputs into 5 (not 6) 128-row tiles:
    [q0:128 | q128:192+k0:64 | k64:192 | v0:128 | v128:192+pad].
  - RoPE rotate-half comes from one extra matmul with a shared 128x128
    +-1 permutation matrix P (contraction 128) instead of duplicated
    rank-contraction weight matmuls.
  - Causal block structure is exploited: the fully-masked
    (keytile1 x querytile0) block is never computed -- not in scores,
    exp, denominators, nor the AV matmul.  E layout per (item, head) is
    [kt0q0 | kt1q1 | kt0q1] so the two triangular diagonal blocks are
    adjacent and share one mask multiply.
  - Softmax denominators: matmuls against an all-ones [128,128]
    stationary produce the denominator already replicated on all 128
    partitions (same column cost as a ones-vector), so the reciprocal
    runs as an efficient full-width [128,512] vector op and no
    partition broadcast is needed.  No DRAM round trip (the fp32
    baseline's 4-hop DRAM chain serialized the pipeline at ~16us/pair).
  - x loads are prefetched one pair ahead on the gpsimd DMA queue;
    output stores ride the sync queue; the output projection of pair
    N-1 is interleaved into pair N's attention to keep the PE busy.
"""

import math
import sys

sys.path.insert(0, "/opt/trn_rl_repo")

import numpy as np
import ml_dtypes


def _to_bf16(a):
    return a.astype(ml_dtypes.bfloat16)


B, T, D = 128, 256, 768
H, HD = 6, 128
RANK = 192
N_CORES = 8
B_LOC = B // N_CORES  # 16
N_PAIRS = B_LOC // 2  # 8 (2 batch items per pipeline iteration)
SCALE = 1.0 / math.sqrt(HD)

_CACHE = {}


def build_program(n_pairs=N_PAIRS):
    import concourse.tile as tile
    from concourse import bacc, mybir
    from contextlib import ExitStack

    f32 = mybir.dt.float32
    bf16 = mybir.dt.bfloat16
    TOK = n_pairs * 512

    nc = bacc.Bacc("TRN2", target_bir_lowering=False, debug=False,
                   num_devices=N_CORES)

    def din(name, shape):
        return nc.dram_tensor(name, shape, bf16, kind="ExternalInput").ap()

    xT = din("xT", [6, 128, TOK])
    Ap_l = din("Ap_l", [6, 128, 640])
    qBp_l = din("qBp_l", [2, 128, 768])
    kBp_l = din("kBp_l", [2, 128, 768])
    vBp_l = din("vBp_l", [2, 128, 768])
    ow_l = din("ow_l", [6, 128, 768])
    P_l = din("P_l", [128, 128])
    cosT = din("cosT", [128, 256])
    sinT = din("sinT", [128, 256])
    tril_l = din("tril_l", [128, 128])
    ones_l = din("ones_l", [128, 128])
    outT = nc.dram_tensor("outT", [6, 128, TOK], f32, kind="ExternalOutput").ap()

    with tile.TileContext(nc) as tc:
        with ExitStack() as ctx:
            wp = ctx.enter_context(tc.tile_pool(name="w", bufs=1))
            xp = ctx.enter_context(tc.tile_pool(name="xt", bufs=2))
            xrp = ctx.enter_context(tc.tile_pool(name="xr", bufs=2))
            rawp = ctx.enter_context(tc.tile_pool(name="raw", bufs=2))
            qkp = ctx.enter_context(tc.tile_pool(name="qk", bufs=1))
            vp_ = ctx.enter_context(tc.tile_pool(name="vsb", bufs=2))
            tp = ctx.enter_context(tc.tile_pool(name="tmp", bufs=2))
            ep = ctx.enter_context(tc.tile_pool(name="eexp", bufs=8))
            dp = ctx.enter_context(tc.tile_pool(name="den", bufs=2))
            bp = ctx.enter_context(tc.tile_pool(name="bcast", bufs=2))
            orp = ctx.enter_context(tc.tile_pool(name="oraw", bufs=2))
            aop = ctx.enter_context(tc.tile_pool(name="ao", bufs=2))
            fp = ctx.enter_context(tc.tile_pool(name="fout", bufs=2))
            ps = ctx.enter_context(tc.tile_pool(name="ps", bufs=3, space="PSUM"))
            pm = ctx.enter_context(tc.tile_pool(name="pm", bufs=2, space="PSUM"))
            pr = ctx.enter_context(tc.tile_pool(name="pr", bufs=1, space="PSUM"))
            spp = ctx.enter_context(tc.tile_pool(name="sp", bufs=2, space="PSUM"))

            def psum():
                return ps.tile([128, 512], f32, tag="ps", name="psb")

            # ---- resident weights / constants (gpsimd DMA queue) ----
            def wload(name, src, shape, perm=None):
                t = wp.tile(shape, bf16, tag=name, name=name)
                nc.gpsimd.dma_start(t[:], src.rearrange(perm) if perm else src)
                return t

            A_s = wload("Ap", Ap_l, [128, 6, 640], "k p m -> p k m")
            qBp_s = wload("qBp", qBp_l, [128, 2, 768], "k p m -> p k m")
            kBp_s = wload("kBp", kBp_l, [128, 2, 768], "k p m -> p k m")
            vBp_s = wload("vBp", vBp_l, [128, 2, 768], "k p m -> p k m")
            ow_s = wload("ow", ow_l, [128, 6, 768], "k p m -> p k m")
            P_s = wload("P", P_l, [128, 128])
            cos_s = wload("cos", cosT, [128, 256])
            sin_s = wload("sin", sinT, [128, 256])
            tril_s = wload("tril", tril_l, [128, 128])
            ones_s = wload("ones", ones_l, [128, 128])

            def emit_outproj(aosb_prev, pr_prev, mts, half=None):
                w = 512 if half is None else 256
                c0 = 0 if half in (None, 0) else 256
                tokp = slice(pr_prev * 512 + c0, pr_prev * 512 + c0 + w)
                for mt in mts:
                    fps = psum()
                    for kt in range(6):
                        nc.tensor.matmul(
                            fps[:, 0:w],
                            ow_s[:, kt, mt * 128:(mt + 1) * 128],
                            aosb_prev[:, kt, c0:c0 + w],
                            start=(kt == 0), stop=(kt == 5))
                    fout = fp.tile([128, 512], f32, tag="fout", name="fout")
                    nc.scalar.copy(fout[:, 0:w], fps[:, 0:w])
                    nc.sync.dma_start(outT[mt, :, tokp], fout[:, 0:w])

            # prefetch first x pair
            xts = [None] * n_pairs

            def load_xt(p):
                t = xp.tile([128, 6, 512], bf16, tag="xt", name="xt")
                nc.gpsimd.dma_start(
                    t[:], xT[:, :, p * 512:(p + 1) * 512].rearrange("k p t -> p k t"))
                xts[p] = t

            load_xt(0)

            prev = None
            for prx in range(n_pairs):
                if prx + 1 < n_pairs:
                    load_xt(prx + 1)
                xt = xts[prx]

                # ---- proj1: packed rank tiles [q|q+k|k|v|v] ----
                xr = xrp.tile([128, 5, 512], bf16, tag="xr", name="xr")
                for rt in range(5):
                    mm = psum()
                    for kt in range(6):
                        nc.tensor.matmul(
                            mm[:],
                            A_s[:, kt, rt * 128:(rt + 1) * 128],
                            xt[:, kt, :],
                            start=(kt == 0), stop=(kt == 5))
                    nc.scalar.copy(xr[:, rt, :], mm[:])

                # ---- proj2 + RoPE for q and k (feature-major) ----
                # q contracts xr tiles {0,1}; k contracts {1,2} (B rows
                # zero-padded on host where tiles are shared).
                qsb = qkp.tile([128, 6, 512], bf16, tag="qsb", name="qsb")
                ksb = qkp.tile([128, 6, 512], bf16, tag="ksb", name="ksb")
                for h in range(6):
                    hc = slice(h * 128, (h + 1) * 128)
                    mains = {}
                    for pname, B_s, t0, sb in (
                            ("q", qBp_s, 0, qsb), ("k", kBp_s, 1, ksb)):
                        p_main = pm.tile([128, 512], f32, tag="pm", name="pm")
                        for kt in range(2):
                            nc.tensor.matmul(
                                p_main[:], B_s[:, kt, hc], xr[:, t0 + kt, :],
                                start=(kt == 0), stop=(kt == 1))
                        raw = rawp.tile([128, 512], bf16, tag=f"raw{pname}",
                                        name=f"raw{pname}")
                        nc.scalar.copy(raw[:], p_main[:])
                        mains[pname] = (p_main, raw, sb)
                    for pname in ("q", "k"):
                        p_main, raw, sb = mains[pname]
                        p_rot = pr.tile([128, 512], f32, tag="pr", name="pr")
                        nc.tensor.matmul(p_rot[:], P_s[:], raw[:],
                                         start=True, stop=True)
                        tmp = tp.tile([128, 512], bf16, tag="ropetmp",
                                      name="ropetmp")
                        nc.vector.tensor_tensor(
                            sb[:, h, :].rearrange("p (b q) -> p b q", b=2),
                            raw[:].rearrange("p (b q) -> p b q", b=2),
                            cos_s[:, None, :].to_broadcast((128, 2, 256)),
                            mybir.AluOpType.mult)
                        nc.vector.tensor_tensor(
                            tmp[:].rearrange("p (b q) -> p b q", b=2),
                            p_rot[:].rearrange("p (b q) -> p b q", b=2),
                            sin_s[:, None, :].to_broadcast((128, 2, 256)),
                            mybir.AluOpType.mult)
                        nc.vector.tensor_tensor(
                            sb[:, h, :], sb[:, h, :], tmp[:],
                            mybir.AluOpType.add)

                # ---- proj2 for v (token-major), contracts xr tiles {3,4} ----
                vsb = vp_.tile([128, 4, 768], bf16, tag="vsb", name="vsb")
                for mt in range(4):
                    for nch in range(2):
                        vps = psum()
                        for kt in range(2):
                            nc.tensor.matmul(
                                vps[:, 0:384],
                                xr[:, 3 + kt, mt * 128:(mt + 1) * 128],
                                vBp_s[:, kt, nch * 384:(nch + 1) * 384],
                                start=(kt == 0), stop=(kt == 1))
                        nc.scalar.copy(vsb[:, mt, nch * 384:(nch + 1) * 384],
                                       vps[:, 0:384])

                # ---- attention (per batch item) ----
                # E layout per (b, h): [kt0q0 | kt1q1 | kt0q1], each 128 wide.
                aosb = aop.tile([128, 6, 512], bf16, tag="aosb", name="aosb")
                for b in range(2):
                    if prev is not None:
                        emit_outproj(prev[0], prev[1], range(3 * b, 3 * b + 3))
                    i_bD = bp.tile([128, 1536], f32, tag="ibD", name="ibD")
                    Es = []
                    for h in range(6):
                        base = h * 512 + 0  # qsb is [128, 6, 512]
                        qh = qsb[:, h, b * 256:(b + 1) * 256]
                        kh = ksb[:, h, b * 256:(b + 1) * 256]
                        sp = spp.tile([128, 512], f32, tag="sp", name="sp")
                        # kt0 stationary shared by the two N=128 matmuls
                        nc.tensor.matmul(sp[:, 0:128], kh[:, 0:128],
                                         qh[:, 0:128], start=True, stop=True)
                        nc.tensor.matmul(sp[:, 256:384], kh[:, 0:128],
                                         qh[:, 128:256], start=True, stop=True)
                        nc.tensor.matmul(sp[:, 128:256], kh[:, 128:256],
                                         qh[:, 128:256], start=True, stop=True)
                        E = ep.tile([128, 384], bf16, tag="E", name="E")
                        nc.scalar.activation(
                            E[:], sp[:, 0:384],
                            mybir.ActivationFunctionType.Exp, scale=SCALE)
                        # mask the two adjacent triangular diagonal blocks
                        nc.vector.tensor_tensor(
                            E[:, 0:256].rearrange("p (x q) -> p x q", x=2),
                            E[:, 0:256].rearrange("p (x q) -> p x q", x=2),
                            tril_s[:, None, :].to_broadcast((128, 2, 128)),
                            mybir.AluOpType.mult)
                        Es.append(E)
                    # denominators: all-ones stationary replicates the key-sum
                    # across all 128 partitions (broadcast comes for free)
                    dslots = []
                    for h in range(6):
                        E = Es[h]
                        s, c = h // 2, (h % 2) * 256
                        if h % 2 == 0:
                            dslots.append(psum())
                        dps = dslots[s]
                        nc.tensor.matmul(dps[:, c:c + 128], ones_s[:],
                                         E[:, 0:128], start=True, stop=True)
                        nc.tensor.matmul(dps[:, c + 128:c + 256], ones_s[:],
                                         E[:, 128:256], start=True, stop=False)
                        nc.tensor.matmul(dps[:, c + 128:c + 256], ones_s[:],
                                         E[:, 256:384], start=False, stop=True)
                        if h % 2 == 1:
                            sc = slice(s * 512, (s + 1) * 512)
                            nc.vector.reciprocal(i_bD[:, sc], dps[:, :])
                    # attention output (feature-major), normalized straight
                    # out of PSUM
                    for h in range(6):
                        E = Es[h]
                        o2 = psum()
                        v0 = vsb[:, b * 2 + 0, h * 128:(h + 1) * 128]
                        v1 = vsb[:, b * 2 + 1, h * 128:(h + 1) * 128]
                        nc.tensor.matmul(o2[:, 0:128], v0, E[:, 0:128],
                                         start=True, stop=True)
                        nc.tensor.matmul(o2[:, 128:256], v0, E[:, 256:384],
                                         start=True, stop=False)
                        nc.tensor.matmul(o2[:, 128:256], v1, E[:, 128:256],
                                         start=False, stop=True)
                        nc.vector.tensor_tensor(
                            aosb[:, h, b * 256:(b + 1) * 256],
                            o2[:, 0:256],
                            i_bD[:, h * 256:(h + 1) * 256],
                            mybir.AluOpType.mult)
                prev = (aosb, prx)

            if prev is not None:
                emit_outproj(prev[0], prev[1], range(6), half=0)
                emit_outproj(prev[0], prev[1], range(6), half=1)

    nc.compile()
    return nc


def _rope_tables():
    inv = 1.0 / (10000.0 ** (np.arange(0, HD, 2, dtype=np.float32) / HD))
    t = np.arange(T, dtype=np.float32)
    freqs = np.outer(t, inv)                      # [T, 64]
    emb = np.concatenate([freqs, freqs], axis=-1)  # [T, 128]
    return np.cos(emb).astype(np.float32), np.sin(emb).astype(np.float32)


def _prep_shared(qA, qB, kA, kB, vA, vB, o_w):
    """Host-side weight/constant layouts (shared by all cores)."""
    def a_r(A):  # [768,192] -> [6,128,192]
        return A.reshape(6, 128, RANK)

    qA_r, kA_r, vA_r = a_r(qA), a_r(kA), a_r(vA)
    Ap = np.zeros((6, 128, 640), np.float32)
    Ap[:, :, 0:128] = qA_r[:, :, 0:128]
    Ap[:, :, 128:192] = qA_r[:, :, 128:192]
    Ap[:, :, 192:256] = kA_r[:, :, 0:64]
    Ap[:, :, 256:384] = kA_r[:, :, 64:192]
    Ap[:, :, 384:512] = vA_r[:, :, 0:128]
    Ap[:, :, 512:576] = vA_r[:, :, 128:192]

    qBp = np.zeros((2, 128, D), np.float32)
    qBp[0] = qB[0:128]
    qBp[1, 0:64] = qB[128:192]

    kBp = np.zeros((2, 128, D), np.float32)
    kBp[0, 64:128] = kB[0:64]
    kBp[1] = kB[64:192]

    vBp = np.zeros((2, 128, D), np.float32)
    vBp[0] = vB[0:128]
    vBp[1, 0:64] = vB[128:192]

    # rotate-half permutation (as matmul lhsT): out[m] = sum_k P[k,m] q[k]
    P = np.zeros((128, 128), np.float32)
    for m in range(64):
        P[m + 64, m] = -1.0
        P[m, m + 64] = 1.0

    cos, sin = _rope_tables()
    cosT = np.ascontiguousarray(cos.T)  # [128, 256]
    sinT = np.ascontiguousarray(sin.T)

    p = np.arange(128)[:, None]
    j = np.arange(128)[None, :]
    tril = (p <= j).astype(np.float32)  # keys (partitions) <= queries (cols)

    return {
        "Ap_l": _to_bf16(np.ascontiguousarray(Ap)),
        "qBp_l": _to_bf16(qBp), "kBp_l": _to_bf16(kBp), "vBp_l": _to_bf16(vBp),
        "ow_l": _to_bf16(np.ascontiguousarray(o_w.reshape(6, 128, D))),
        "P_l": _to_bf16(P),
        "cosT": _to_bf16(cosT), "sinT": _to_bf16(sinT),
        "tril_l": _to_bf16(tril),
        "ones_l": _to_bf16(np.ones((128, 128), np.float32)),
    }


def x_to_xT(xc):
    """[b, T, D] -> [6, 128, b*T] feature-major, batch-major tokens."""
    nb = xc.shape[0]
    return np.ascontiguousarray(
        _to_bf16(xc).reshape(nb, T, 6, 128).transpose(2, 3, 0, 1).reshape(6, 128, nb * T))


def outT_to_out(oT, nb):
    return np.ascontiguousarray(
        oT.reshape(6, 128, nb, T).transpose(2, 3, 0, 1).reshape(nb, T, D))


def kernel(x, qA, qB, kA, kB, vA, vB, o_w):
    from concourse import bass_utils

    if "nc" not in _CACHE:
        _CACHE["nc"] = build_program(N_PAIRS)
    nc = _CACHE["nc"]

    shared = _prep_shared(
        np.asarray(qA, np.float32), np.asarray(qB, np.float32),
        np.asarray(kA, np.float32), np.asarray(kB, np.float32),
        np.asarray(vA, np.float32), np.asarray(vB, np.float32),
        np.asarray(o_w, np.float32))
    x = np.asarray(x, np.float32)

    in_maps = []
    for c in range(N_CORES):
        m = dict(shared)
        m["xT"] = x_to_xT(x[c * B_LOC:(c + 1) * B_LOC])
        in_maps.append(m)

    res = bass_utils.run_bass_kernel_spmd(
        nc, in_maps, core_ids=list(range(N_CORES)))
    out = np.empty((B, T, D), np.float32)
    for c in range(N_CORES):
        out[c * B_LOC:(c + 1) * B_LOC] = outT_to_out(
            res.results[c]["outT"], B_LOC)
    return out


# revision 17
# speedup vs baseline: 2.2458x; 1.4239x over previous
"""Trainium2 Bass kernel for nn_Attn_30623116820602.

Low-rank-projected causal multi-head attention:
  q/k/v = (x @ A) @ B  (rank 192), RoPE on q,k, causal softmax attention,
  output projection.  x: [128, 256, 768] fp32.

Sharding: pure data-parallel over batch (16 items per core, 8 cores).
Feature-major layout (d_model on partitions) throughout; host pre/post
transposes.  All matmul inputs are bf16 (PSUM accumulates fp32).

Structure (per pair of batch items = 512 token columns):
  - proj1 packs the 3 rank-192 outputs into 5 (not 6) 128-row tiles:
    [q0:128 | q128:192+k0:64 | k64:192 | v0:128 | v128:192+pad].
  - RoPE rotate-half comes from one extra matmul with a shared 128x128
    +-1 permutation matrix P (contraction 128) instead of duplicated
    rank-contraction weight matmuls.
  - Causal block structure is exploited: the fully-masked
    (keytile1 x querytile0) block is never computed -- not in scores,
    exp, denominators, nor the AV matmul.  E layout per (item, head) is
    [kt0q0 | kt1q1 | kt0q1] so the two triangular diagonal blocks are
    adjacent and share one mask multiply.
  - Softmax denominators: matmuls against an all-ones [128,128]
    stationary produce the denominator already replicated on all 128
    partitions (same column cost as a ones-vector), so the reciprocal
    runs as an efficient full-width [128,512] vector op and no
    partition broadcast is needed.  No DRAM round trip (the fp32
    baseline's 4-hop DRAM chain serialized the pipeline at ~16us/pair).
  - x loads are prefetched one pair ahead on the gpsimd DMA queue;
    output stores ride the sync queue; the output projection of pair
    N-1 is interleaved into pair N's attention to keep the PE busy.
"""

import math
import sys

sys.path.insert(0, "/opt/trn_rl_repo")

import numpy as np
import ml_dtypes


def _to_bf16(a):
    return a.astype(ml_dtypes.bfloat16)


B, T, D = 128, 256, 768
H, HD = 6, 128
RANK = 192
N_CORES = 8
B_LOC = B // N_CORES  # 16
N_PAIRS = B_LOC // 2  # 8 (2 batch items per pipeline iteration)
SCALE = 1.0 / math.sqrt(HD)

_CACHE = {}


def build_program(n_pairs=N_PAIRS):
    import concourse.tile as tile
    from concourse import bacc, mybir
    from contextlib import ExitStack

    f32 = mybir.dt.float32
    bf16 = mybir.dt.bfloat16
    TOK = n_pairs * 512

    nc = bacc.Bacc("TRN2", target_bir_lowering=False, debug=False,
                   num_devices=N_CORES)

    def din(name, shape):
        return nc.dram_tensor(name, shape, bf16, kind="ExternalInput").ap()

    xT = din("xT", [6, 128, TOK])
    Ap_l = din("Ap_l", [6, 128, 640])
    qBp_l = din("qBp_l", [2, 128, 768])
    kBp_l = din("kBp_l", [2, 128, 768])
    vBp_l = din("vBp_l", [2, 128, 768])
    ow_l = din("ow_l", [6, 128, 768])
    P_l = din("P_l", [128, 128])
    cosT = din("cosT", [128, 256])
    sinT = din("sinT", [128, 256])
    mask_l = din("mask_l", [128, 384])
    ones_l = din("ones_l", [128, 128])
    outT = nc.dram_tensor("outT", [6, 128, TOK], f32, kind="ExternalOutput").ap()

    with tile.TileContext(nc) as tc:
        with ExitStack() as ctx:
            wp = ctx.enter_context(tc.tile_pool(name="w", bufs=1))
            xp = ctx.enter_context(tc.tile_pool(name="xt", bufs=2))
            xrp = ctx.enter_context(tc.tile_pool(name="xr", bufs=2))
            rawp = ctx.enter_context(tc.tile_pool(name="raw", bufs=2))
            qkp = ctx.enter_context(tc.tile_pool(name="qk", bufs=1))
            vp_ = ctx.enter_context(tc.tile_pool(name="vsb", bufs=2))
            tp = ctx.enter_context(tc.tile_pool(name="tmp", bufs=2))
            ep = ctx.enter_context(tc.tile_pool(name="eexp", bufs=8))
            dp = ctx.enter_context(tc.tile_pool(name="den", bufs=2))
            bp = ctx.enter_context(tc.tile_pool(name="bcast", bufs=2))
            orp = ctx.enter_context(tc.tile_pool(name="oraw", bufs=2))
            aop = ctx.enter_context(tc.tile_pool(name="ao", bufs=2))
            fp = ctx.enter_context(tc.tile_pool(name="fout", bufs=2))
            ps = ctx.enter_context(tc.tile_pool(name="ps", bufs=3, space="PSUM"))
            pm = ctx.enter_context(tc.tile_pool(name="pm", bufs=2, space="PSUM"))
            pr = ctx.enter_context(tc.tile_pool(name="pr", bufs=1, space="PSUM"))
            spp = ctx.enter_context(tc.tile_pool(name="sp", bufs=2, space="PSUM"))

            def psum():
                return ps.tile([128, 512], f32, tag="ps", name="psb")

            # ---- resident weights / constants (gpsimd DMA queue) ----
            def wload(name, src, shape, perm=None):
                t = wp.tile(shape, bf16, tag=name, name=name)
                nc.gpsimd.dma_start(t[:], src.rearrange(perm) if perm else src)
                return t

            A_s = wload("Ap", Ap_l, [128, 6, 640], "k p m -> p k m")
            qBp_s = wload("qBp", qBp_l, [128, 2, 768], "k p m -> p k m")
            kBp_s = wload("kBp", kBp_l, [128, 2, 768], "k p m -> p k m")
            vBp_s = wload("vBp", vBp_l, [128, 2, 768], "k p m -> p k m")
            ow_s = wload("ow", ow_l, [128, 6, 768], "k p m -> p k m")
            P_s = wload("P", P_l, [128, 128])
            cos_s = wload("cos", cosT, [128, 256])
            sin_s = wload("sin", sinT, [128, 256])
            mask_s = wload("mask", mask_l, [128, 384])
            ones_s = wload("ones", ones_l, [128, 128])

            def emit_outproj(aosb_prev, pr_prev, mts, half=None):
                w = 512 if half is None else 256
                c0 = 0 if half in (None, 0) else 256
                tokp = slice(pr_prev * 512 + c0, pr_prev * 512 + c0 + w)
                for mt in mts:
                    fps = psum()
                    for kt in range(6):
                        nc.tensor.matmul(
                            fps[:, 0:w],
                            ow_s[:, kt, mt * 128:(mt + 1) * 128],
                            aosb_prev[:, kt, c0:c0 + w],
                            start=(kt == 0), stop=(kt == 5))
                    fout = fp.tile([128, 512], f32, tag="fout", name="fout")
                    nc.scalar.copy(fout[:, 0:w], fps[:, 0:w])
                    nc.sync.dma_start(outT[mt, :, tokp], fout[:, 0:w])

            # prefetch first x pair
            xts = [None] * n_pairs

            def load_xt(p):
                t = xp.tile([128, 6, 512], bf16, tag="xt", name="xt")
                nc.gpsimd.dma_start(
                    t[:], xT[:, :, p * 512:(p + 1) * 512].rearrange("k p t -> p k t"))
                xts[p] = t

            load_xt(0)

            prev = None
            for prx in range(n_pairs):
                if prx + 1 < n_pairs:
                    load_xt(prx + 1)
                xt = xts[prx]

                # ---- proj1: packed rank tiles [q|q+k|k|v|v] ----
                xr = xrp.tile([128, 5, 512], bf16, tag="xr", name="xr")
                for rt in range(5):
                    mm = psum()
                    for kt in range(6):
                        nc.tensor.matmul(
                            mm[:],
                            A_s[:, kt, rt * 128:(rt + 1) * 128],
                            xt[:, kt, :],
                            start=(kt == 0), stop=(kt == 5))
                    nc.scalar.copy(xr[:, rt, :], mm[:])

                # ---- proj2 + RoPE for q and k (feature-major) ----
                # q contracts xr tiles {0,1}; k contracts {1,2} (B rows
                # zero-padded on host where tiles are shared).
                qsb = qkp.tile([128, 6, 512], bf16, tag="qsb", name="qsb")
                ksb = qkp.tile([128, 6, 512], bf16, tag="ksb", name="ksb")
                for h in range(6):
                    hc = slice(h * 128, (h + 1) * 128)
                    mains = {}
                    for pname, B_s, t0, sb in (
                            ("q", qBp_s, 0, qsb), ("k", kBp_s, 1, ksb)):
                        p_main = pm.tile([128, 512], f32, tag="pm", name="pm")
                        for kt in range(2):
                            nc.tensor.matmul(
                                p_main[:], B_s[:, kt, hc], xr[:, t0 + kt, :],
                                start=(kt == 0), stop=(kt == 1))
                        raw = rawp.tile([128, 512], bf16, tag=f"raw{pname}",
                                        name=f"raw{pname}")
                        nc.scalar.copy(raw[:], p_main[:])
                        mains[pname] = (p_main, raw, sb)
                    for pname in ("q", "k"):
                        p_main, raw, sb = mains[pname]
                        p_rot = pr.tile([128, 512], f32, tag="pr", name="pr")
                        nc.tensor.matmul(p_rot[:], P_s[:], raw[:],
                                         start=True, stop=True)
                        tmp = tp.tile([128, 512], bf16, tag="ropetmp",
                                      name="ropetmp")
                        nc.vector.tensor_tensor(
                            sb[:, h, :].rearrange("p (b q) -> p b q", b=2),
                            raw[:].rearrange("p (b q) -> p b q", b=2),
                            cos_s[:, None, :].to_broadcast((128, 2, 256)),
                            mybir.AluOpType.mult)
                        nc.vector.tensor_tensor(
                            tmp[:].rearrange("p (b q) -> p b q", b=2),
                            p_rot[:].rearrange("p (b q) -> p b q", b=2),
                            sin_s[:, None, :].to_broadcast((128, 2, 256)),
                            mybir.AluOpType.mult)
                        nc.vector.tensor_tensor(
                            sb[:, h, :], sb[:, h, :], tmp[:],
                            mybir.AluOpType.add)

                # ---- proj2 for v (token-major), contracts xr tiles {3,4} ----
                vsb = vp_.tile([128, 4, 768], bf16, tag="vsb", name="vsb")
                for mt in range(4):
                    for nch in range(2):
                        vps = psum()
                        for kt in range(2):
                            nc.tensor.matmul(
                                vps[:, 0:384],
                                xr[:, 3 + kt, mt * 128:(mt + 1) * 128],
                                vBp_s[:, kt, nch * 384:(nch + 1) * 384],
                                start=(kt == 0), stop=(kt == 1))
                        nc.scalar.copy(vsb[:, mt, nch * 384:(nch + 1) * 384],
                                       vps[:, 0:384])

                # ---- attention (per batch item) ----
                # E layout per (b, h): [kt0q0 | kt0q1 | kt1q1], each 128 wide.
                aosb = aop.tile([128, 6, 512], bf16, tag="aosb", name="aosb")
                for b in range(2):
                    if prev is not None:
                        emit_outproj(prev[0], prev[1], range(3 * b, 3 * b + 3))
                    i_bD = bp.tile([128, 1536], f32, tag="ibD", name="ibD")
                    Es = []
                    for h in range(6):
                        base = h * 512 + 0  # qsb is [128, 6, 512]
                        qh = qsb[:, h, b * 256:(b + 1) * 256]
                        kh = ksb[:, h, b * 256:(b + 1) * 256]
                        sp = spp.tile([128, 512], f32, tag="sp", name="sp")
                        nc.tensor.matmul(sp[:, 0:256], kh[:, 0:128],
                                         qh[:, 0:256], start=True, stop=True)
                        nc.tensor.matmul(sp[:, 256:384], kh[:, 128:256],
                                         qh[:, 128:256], start=True, stop=True)
                        E = ep.tile([128, 384], bf16, tag="E", name="E")
                        nc.scalar.activation(
                            E[:], sp[:, 0:384],
                            mybir.ActivationFunctionType.Exp, scale=SCALE)
                        # one mask multiply: [tril | ones | tril]
                        nc.vector.tensor_tensor(
                            E[:], E[:], mask_s[:],
                            mybir.AluOpType.mult)
                        Es.append(E)
                    # denominators: all-ones stationary replicates the key-sum
                    # across all 128 partitions (broadcast comes for free)
                    dslots = []
                    for h in range(6):
                        E = Es[h]
                        s, c = h // 2, (h % 2) * 256
                        if h % 2 == 0:
                            dslots.append(psum())
                        dps = dslots[s]
                        nc.tensor.matmul(dps[:, c:c + 256], ones_s[:],
                                         E[:, 0:256], start=True, stop=False,
                                         skip_group_check=True)
                        nc.tensor.matmul(dps[:, c + 128:c + 256], ones_s[:],
                                         E[:, 256:384], start=False, stop=True,
                                         skip_group_check=True)
                        if h % 2 == 1:
                            sc = slice(s * 512, (s + 1) * 512)
                            nc.vector.reciprocal_approx_fast(
                                i_bD[:, sc], dps[:, :])
                    # attention output (feature-major), normalized straight
                    # out of PSUM
                    for h in range(6):
                        E = Es[h]
                        o2 = psum()
                        v0 = vsb[:, b * 2 + 0, h * 128:(h + 1) * 128]
                        v1 = vsb[:, b * 2 + 1, h * 128:(h + 1) * 128]
                        nc.tensor.matmul(o2[:, 0:256], v0, E[:, 0:256],
                                         start=True, stop=False,
                                         skip_group_check=True)
                        nc.tensor.matmul(o2[:, 128:256], v1, E[:, 256:384],
                                         start=False, stop=True,
                                         skip_group_check=True)
                        nc.vector.tensor_tensor(
                            aosb[:, h, b * 256:(b + 1) * 256],
                            o2[:, 0:256],
                            i_bD[:, h * 256:(h + 1) * 256],
                            mybir.AluOpType.mult)
                prev = (aosb, prx)

            if prev is not None:
                emit_outproj(prev[0], prev[1], range(6), half=0)
                emit_outproj(prev[0], prev[1], range(6), half=1)

    nc.compile()
    return nc


def _rope_tables():
    inv = 1.0 / (10000.0 ** (np.arange(0, HD, 2, dtype=np.float32) / HD))
    t = np.arange(T, dtype=np.float32)
    freqs = np.outer(t, inv)                      # [T, 64]
    emb = np.concatenate([freqs, freqs], axis=-1)  # [T, 128]
    return np.cos(emb).astype(np.float32), np.sin(emb).astype(np.float32)


def _prep_shared(qA, qB, kA, kB, vA, vB, o_w):
    """Host-side weight/constant layouts (shared by all cores)."""
    def a_r(A):  # [768,192] -> [6,128,192]
        return A.reshape(6, 128, RANK)

    qA_r, kA_r, vA_r = a_r(qA), a_r(kA), a_r(vA)
    Ap = np.zeros((6, 128, 640), np.float32)
    Ap[:, :, 0:128] = qA_r[:, :, 0:128]
    Ap[:, :, 128:192] = qA_r[:, :, 128:192]
    Ap[:, :, 192:256] = kA_r[:, :, 0:64]
    Ap[:, :, 256:384] = kA_r[:, :, 64:192]
    Ap[:, :, 384:512] = vA_r[:, :, 0:128]
    Ap[:, :, 512:576] = vA_r[:, :, 128:192]

    qBp = np.zeros((2, 128, D), np.float32)
    qBp[0] = qB[0:128]
    qBp[1, 0:64] = qB[128:192]

    kBp = np.zeros((2, 128, D), np.float32)
    kBp[0, 64:128] = kB[0:64]
    kBp[1] = kB[64:192]

    vBp = np.zeros((2, 128, D), np.float32)
    vBp[0] = vB[0:128]
    vBp[1, 0:64] = vB[128:192]

    # rotate-half permutation (as matmul lhsT): out[m] = sum_k P[k,m] q[k]
    P = np.zeros((128, 128), np.float32)
    for m in range(64):
        P[m + 64, m] = -1.0
        P[m, m + 64] = 1.0

    cos, sin = _rope_tables()
    cosT = np.ascontiguousarray(cos.T)  # [128, 256]
    sinT = np.ascontiguousarray(sin.T)

    p = np.arange(128)[:, None]
    j = np.arange(128)[None, :]
    tril = (p <= j).astype(np.float32)  # keys (partitions) <= queries (cols)
    mask = np.concatenate(
        [tril, np.ones((128, 128), np.float32), tril], axis=1)  # [128, 384]

    return {
        "Ap_l": _to_bf16(np.ascontiguousarray(Ap)),
        "qBp_l": _to_bf16(qBp), "kBp_l": _to_bf16(kBp), "vBp_l": _to_bf16(vBp),
        "ow_l": _to_bf16(np.ascontiguousarray(o_w.reshape(6, 128, D))),
        "P_l": _to_bf16(P),
        "cosT": _to_bf16(cosT), "sinT": _to_bf16(sinT),
        "mask_l": _to_bf16(mask),
        "ones_l": _to_bf16(np.ones((128, 128), np.float32)),
    }


def x_to_xT(xc):
    """[b, T, D] -> [6, 128, b*T] feature-major, batch-major tokens."""
    nb = xc.shape[0]
    return np.ascontiguousarray(
        _to_bf16(xc).reshape(nb, T, 6, 128).transpose(2, 3, 0, 1).reshape(6, 128, nb * T))


def outT_to_out(oT, nb):
    return np.ascontiguousarray(
        oT.reshape(6, 128, nb, T).transpose(2, 3, 0, 1).reshape(nb, T, D))


def kernel(x, qA, qB, kA, kB, vA, vB, o_w):
    from concourse import bass_utils

    if "nc" not in _CACHE:
        _CACHE["nc"] = build_program(N_PAIRS)
    nc = _CACHE["nc"]

    shared = _prep_shared(
        np.asarray(qA, np.float32), np.asarray(qB, np.float32),
        np.asarray(kA, np.float32), np.asarray(kB, np.float32),
        np.asarray(vA, np.float32), np.asarray(vB, np.float32),
        np.asarray(o_w, np.float32))
    x = np.asarray(x, np.float32)

    in_maps = []
    for c in range(N_CORES):
        m = dict(shared)
        m["xT"] = x_to_xT(x[c * B_LOC:(c + 1) * B_LOC])
        in_maps.append(m)

    res = bass_utils.run_bass_kernel_spmd(
        nc, in_maps, core_ids=list(range(N_CORES)))
    out = np.empty((B, T, D), np.float32)
    for c in range(N_CORES):
        out[c * B_LOC:(c + 1) * B_LOC] = outT_to_out(
            res.results[c]["outT"], B_LOC)
    return out
